# revision 20
# baseline (speedup 1.0000x reference)
"""LIF neuron scan kernel for Trainium2, sharded over 8 NeuronCores.

Device computes the membrane trajectory with ONE fused custom-DVE
instruction per time step (a microprogrammed 8-slice ALU chain, table
written per-NEFF — no firmware change):

    u_t = select(u_{t-1} < 1, u_{t-1}, 0) * 0.95 + x_t

i.e. the state is carried as the pre-reset potential u, and the reset,
decay and input-add all happen inside one DVE pass (1 elem/cycle/lane,
~242ns per [128,128] step tile vs 2-3 stock instructions).

The decay uses a single-rounded 0.95 multiply; the reference rounds twice
(v - v*0.05). Measured divergence ("drift") of the device trajectory from
the exact one is <= ~4e-6 (differences reset to zero whenever both
trajectories spike together). Host post-pass repairs any possible spike
flips — see the q-code scheme below.

Output: the otherwise-idle ACT engine quantizes u to a uint8 near-threshold
code per chunk (round-to-nearest, saturating — HW-verified):

    q = sat_u8(K*(u - 1) + 128),  K = 16384

so q >= 128  <=>  u >= 1 - 0.5/K   (the spike bit, exact outside the band)
and q in [125, 131]  <=>  |u - 1| <~ 2.1e-4  (the repair band, 50x drift).
This cuts output DMA 4x (26.2MB -> 6.55MB per core), putting total HBM
traffic at 32.8MB/core, under the ~358GB/s HBM-per-core roofline at the
~120us target. Host: spikes = (q >= 128); rows with any q in [125,131]
are re-simulated exactly (numpy f32, reference op order) and replaced —
~1-3% of rows.

Sharding: batch dim B=131072 split into 8 contiguous blocks of 16384 rows.
Per core the block is laid out time-major as [128 partitions, 400 steps,
128 neurons]; each step is one [128,128] SBUF tile; u is written in place
over the input tile (the previous step's tile is the recurrence input).

Raw Bass (no TileContext), semaphores at per-chunk granularity:
  dma_in   (+16 per input DMA, sync queue)   gates DVE's first read
  dma_in2  (+16 per input DMA, scalar queue) same, for the early chunk
                                             issued from the scalar queue
  dve_done (+1 per chunk)   gates the ACT quantize
  act_done (+1 per chunk)   gates the output DMA (dma_start is
                            sequencer-only and does NOT order after
                            same-engine compute writes) + xin reuse
  dma_out  (+16 per output DMA, scalar queue) gates sout reuse + kernel end
All intra-engine ordering is program order (engines execute in-order).
"""

import numpy as np

import concourse.bacc as bacc
import concourse.mybir as mybir
from concourse.bass_utils import run_bass_kernel_spmd

B, L = 131072, 400
NCORES = 8
RPC = B // NCORES      # rows (neurons) per core
P = 128                # SBUF partitions
J = RPC // P           # neurons per partition = 128 (one step = [P, J] tile)
# Chunk schedule: small first chunks to fill the pipe fast, small last to
# drain fast. Sums to L.
# Big body chunks: each chunk boundary exposes the ~2.3us DMA completion
# ack on the DVE critical path (the input stream's transfer time per chunk
# nearly equals the DVE chunk time — the kernel sits at the HBM roofline —
# so there is no pipeline slack to absorb it). 60-step chunks cut the
# number of exposures and amortize transfer overheads; small first chunks
# fill the pipe fast, small last drain fast. Body-chunk input DMAs are
# additionally split A(16 steps)+B(rest) with separate waits, so the
# chunk-start wait covers only the small leading transfer and B's ack
# hides behind the first 16 steps of compute.
CHUNKS = [4, 8, 12, 16, 20] + [60] * 5 + [24, 12, 4]
assert sum(CHUNKS) == L
TCMAX = max(CHUNKS)
SUBA = 16              # leading sub-DMA steps for chunks with TC > SUBA+8
NBUF = 4               # in/out chunk buffers (60-step tiles: 4x30KB xin +
                       # 4x7.5KB sout = 150KB/partition of SBUF)
# Chunk 1's input DMA issues from the (initially idle) scalar queue so its
# descriptor generation overlaps chunk 0's on the sync queue.
SCALAR_IN_CHUNKS = {1}

DECAY = 0.95           # single-rounded decay multiplier (host repairs)
QK = 16384.0           # u8 code scale: q = sat_u8(QK*(u-1) + 128)
QLO, QHI = 125, 131    # repair band in q units (|u-1| <~ 2.1e-4)

_nc_cache = None
_lif_op = None


def _get_lif_op():
    """Register the fused LIF step as a custom DVE op (idempotent)."""
    global _lif_op
    if _lif_op is not None:
        return _lif_op
    from concourse.dve_ops import (
        CUSTOM_DVE_SPECS,
        OPS,
        _SUB_OPCODE_FOR_NAME,
        DveOp,
    )
    from concourse.dve_spec import C0, One, Spec, Src0, Src1, Zero, lower, select
    from concourse.dve_uop import DveOpSpec

    name = "LIF_STEP_ANT"
    if name in _SUB_OPCODE_FOR_NAME:
        _lif_op = next(op for op in OPS if op.name == name)
        return _lif_op
    spec = Spec(
        body=select(Src0 < One, Src0, Zero) * C0 + Src1,
        reference=lambda in0, in1, s0, s1, imm2: np.where(in0 < 1.0, in0, 0.0)
        .astype(np.float32)
        * np.float32(s0)
        + in1,
    )
    row = max(_SUB_OPCODE_FOR_NAME.values()) + 1
    assert row < 0x20
    shas = {}
    for ver in ("v3", "v4"):
        try:
            s = DveOpSpec(name=name, opcode=row, uops=lower(spec, ver=ver), rd1_en=True)
            shas[ver] = s.sha(ver)
        except Exception:
            pass
    op = DveOp(name, spec, subdim=False, uops_sha=shas)
    OPS.append(op)
    _SUB_OPCODE_FOR_NAME[name] = row
    CUSTOM_DVE_SPECS[name] = spec
    _lif_op = op
    return op


def _build():
    lif = _get_lif_op()
    nc = bacc.Bacc(None, target_bir_lowering=False)
    X = nc.dram_tensor("X", [P, L * J], mybir.dt.float32, kind="ExternalInput")
    S = nc.dram_tensor("S", [P, L * J], mybir.dt.uint8, kind="ExternalOutput")

    f32 = mybir.dt.float32
    xin = [nc.alloc_sbuf_tensor(f"xin{i}", [P, TCMAX * J], f32) for i in range(NBUF)]
    sout = [
        nc.alloc_sbuf_tensor(f"sout{i}", [P, TCMAX * J], mybir.dt.uint8)
        for i in range(NBUF)
    ]
    zt = nc.alloc_sbuf_tensor("zt", [P, J], f32)   # u_{-1} = 0

    # Input-DMA completion tracking: a +16-per-DMA cumulative count is only
    # sound if same-semaphore DMAs complete in order (the 16 SDMA engines
    # finish their partition slices independently, so two in-flight DMAs on
    # one count can interleave incs and a wait on 16*n passes with DMA n
    # still draining on a laggard engine — observed as stale x on single
    # partitions). Chunks rotate over NSLOT semaphores; each slot chains on
    # its own previous DMA, bounding in-flight DMAs per slot to 1 (exact
    # count) while keeping NSLOT transfers in flight overall.
    # 2 slots: enough overlap to hide the ~2.3us per-DMA completion ack
    # (cadence ~3us < the 4.8us DVE chunk pace) while capping concurrent
    # input streams — 4 slots measurably slowed the DVE (~+40ns/op) via
    # SBUF write-port contention from the extra in-flight transfers.
    NSLOT = 2
    sem_in = [nc.alloc_semaphore(f"dma_slot{k}") for k in range(NSLOT)]
    sem_dma_in2 = nc.alloc_semaphore("dma_in2")   # scalar-queue input DMAs
    sem_dve = nc.alloc_semaphore("dve_done")
    sem_act = nc.alloc_semaphore("act_done")
    # Output-DMA completions use the same slot-pool scheme as inputs (2
    # slots): per-slot self-chaining keeps each cumulative count exact
    # while letting two transfers overlap, and the chain wait (own slot's
    # previous DMA, 2 chunks back) is long satisfied by the time the ACT
    # sequencer reaches it — no stall.
    NOSLOT = 2
    sem_out = [nc.alloc_semaphore(f"dma_oslot{k}") for k in range(NOSLOT)]

    # Chunk base offsets (in steps).
    bases = []
    t0 = 0
    for tc in CHUNKS:
        bases.append(t0)
        t0 += tc

    # --- input DMAs, NBUF-deep rolling prefetch --------------------------
    # DMA for chunk c overwrites xin[c%NBUF]; its last readers are chunk
    # c-NBUF's ACT quantize and chunk c-NBUF+1's first LIF op (which reads
    # chunk c-NBUF's final u tile).
    assert all(c < NBUF for c in SCALAR_IN_CHUNKS)
    in_rank = {}      # chunk -> list of (sem, cumulative wait value, step)
    slot_cnt = [0] * NSLOT
    nsc = 0
    nseq = 0          # sequential index over sync-queue input DMAs

    def _issue(dst, src, s):
        if slot_cnt[s] > 0:
            # Chain on this slot's previous DMA (exact per-slot count).
            nc.sync.wait_ge(sem_in[s], 16 * slot_cnt[s])
        slot_cnt[s] += 1
        nc.sync.dma_start(dst, src).then_inc(sem_in[s], 16)
        return (sem_in[s], 16 * slot_cnt[s])

    for c, TC in enumerate(CHUNKS):
        base = bases[c] * J
        if c in SCALAR_IN_CHUNKS:
            nsc += 1
            in_rank[c] = [(sem_dma_in2, 16 * nsc, 0)]
            nc.scalar.dma_start(
                xin[c % NBUF][:, : TC * J], X[:, base : base + TC * J]
            ).then_inc(sem_dma_in2, 16)
            continue
        if c >= NBUF:
            # xin[c%NBUF] reuse gates.
            nc.sync.wait_ge(sem_act, c - NBUF + 1)
            nc.sync.wait_ge(sem_dve, c - NBUF + 2)
        xb = xin[c % NBUF]
        if TC > SUBA + 8:
            sa, va = _issue(
                xb[:, : SUBA * J], X[:, base : base + SUBA * J], nseq % NSLOT
            )
            nseq += 1
            sb_, vb = _issue(
                xb[:, SUBA * J : TC * J],
                X[:, base + SUBA * J : base + TC * J],
                nseq % NSLOT,
            )
            nseq += 1
            in_rank[c] = [(sa, va, 0), (sb_, vb, SUBA)]
        else:
            s_, v_ = _issue(
                xb[:, : TC * J], X[:, base : base + TC * J], nseq % NSLOT
            )
            nseq += 1
            in_rank[c] = [(s_, v_, 0)]

    # --- DVE queue: the full recurrence, one fused op per step -----------
    nc.vector.memset(zt[:], 0.0)

    prev = zt[:, :]
    for c, TC in enumerate(CHUNKS):
        xb = xin[c % NBUF]
        waits = {step: (ws, wv) for ws, wv, step in in_rank[c]}
        for t in range(TC):
            sl = slice(t * J, (t + 1) * J)
            if t in waits:
                # Gate reads on the covering input (sub-)DMA.
                ws, wv = waits[t]
                nc.vector.wait_ge(ws, wv)
            # u_t = select(u_{t-1} < 1, u_{t-1}, 0) * 0.95 + x_t  (in place)
            ri = nc.vector._custom_dve(
                lif, out=xb[:, sl], in0=prev, in1=xb[:, sl], s0=DECAY
            )
            prev = xb[:, sl]
        # Last DVE op of the chunk certifies all u tiles of xin[c%NBUF].
        ri.then_inc(sem_dve, 1)

    # --- ACT queue: u8 quantize + output DMA -----------------------------
    ocnt = [0] * NOSLOT
    orank = {}        # chunk -> (slot, count)
    for c, TC in enumerate(CHUNKS):
        xb = xin[c % NBUF]
        sb = sout[c % NBUF]
        nc.scalar.wait_ge(sem_dve, c + 1)
        if c >= NBUF:
            # sout[c%NBUF] reuse: chunk c-NBUF's out-DMA must have drained.
            ps, pk = orank[c - NBUF]
            nc.scalar.wait_ge(sem_out[ps], 16 * pk)
        nc.scalar.activation(
            sb[:, : TC * J],
            xb[:, : TC * J],
            mybir.ActivationFunctionType.Copy,
            bias=128.0 - QK,
            scale=QK,
        ).then_inc(sem_act, 1)
        nc.scalar.wait_ge(sem_act, c + 1)
        s = c % NOSLOT
        if ocnt[s] > 0:
            # Chain on this slot's previous DMA (exact per-slot count).
            nc.scalar.wait_ge(sem_out[s], 16 * ocnt[s])
        ocnt[s] += 1
        orank[c] = (s, ocnt[s])
        base = bases[c] * J
        nc.scalar.dma_start(S[:, base : base + TC * J], sb[:, : TC * J]).then_inc(
            sem_out[s], 16
        )

    # Hold kernel end until the last output DMAs drained, then zero the
    # semaphores so back-to-back NEFF executions see a clean file.
    for s in range(NOSLOT):
        nc.sync.wait_ge(sem_out[s], 16 * ocnt[s])
    for s in (*sem_in, sem_dma_in2, sem_dve, sem_act, *sem_out):
        nc.sync.sem_clear(s)

    nc.compile()
    return nc


def _get_nc():
    global _nc_cache
    if _nc_cache is None:
        _nc_cache = _build()
    return _nc_cache


def _shard(I):
    # Per-core host transposes run in parallel (numpy releases the GIL
    # during the strided copies).
    from concurrent.futures import ThreadPoolExecutor

    def one(c):
        Ic = I[c * RPC : (c + 1) * RPC]                    # [RPC, L]
        Xc = Ic.reshape(P, J, L).transpose(0, 2, 1)        # [P, L, J] time-major
        return {"X": np.ascontiguousarray(Xc).reshape(P, L * J)}

    with ThreadPoolExecutor(NCORES) as ex:
        return list(ex.map(one, range(NCORES)))


def _unshard(results):
    from concurrent.futures import ThreadPoolExecutor

    out = np.empty((B, L), np.uint8)

    def one(c):
        Sc = results[c]["S"].reshape(P, L, J).transpose(0, 2, 1)   # [P, J, L]
        out[c * RPC : (c + 1) * RPC] = Sc.reshape(RPC, L)

    with ThreadPoolExecutor(NCORES) as ex:
        list(ex.map(one, range(NCORES)))
    return out


def _resim_exact(Irows):
    """Exact reference recurrence (numpy f32, same rounding order as the
    jax/XLA raster: u = (v - v*0.05) + I_t)."""
    n = Irows.shape[0]
    v = np.zeros(n, np.float32)
    s = np.empty((n, L), np.float32)
    c05 = np.float32(0.05)
    one = np.float32(1.0)
    for t in range(L):
        u = (v - v * c05) + Irows[:, t]
        spk = u >= one
        s[:, t] = spk
        v = np.where(spk, np.float32(0.0), u)
    return s


def _decode(q, I):
    """q codes -> spike raster, with exact re-simulation of band rows."""
    from concurrent.futures import ThreadPoolExecutor

    out = np.empty((B, L), np.float32)
    flagged = []

    def one(c):
        lo, hi = c * RPC, (c + 1) * RPC
        qc = q[lo:hi]
        out[lo:hi] = (qc >= 128).astype(np.float32)
        near = (qc >= QLO) & (qc <= QHI)
        return np.nonzero(near.any(axis=1))[0] + lo

    with ThreadPoolExecutor(NCORES) as ex:
        for rows in ex.map(one, range(NCORES)):
            flagged.append(rows)
    flagged = np.concatenate(flagged)
    if flagged.size:
        out[flagged] = _resim_exact(I[flagged])
    return out, flagged


def kernel(I, _trace=False, _debug=False):
    I = np.ascontiguousarray(np.asarray(I), dtype=np.float32)
    assert I.shape == (B, L), I.shape
    nc = _get_nc()
    br = run_bass_kernel_spmd(nc, _shard(I), core_ids=list(range(NCORES)), trace=_trace)
    q = _unshard(br.results)
    out, flagged = _decode(q, I)
    if _debug:
        return out, q, flagged
    if _trace:
        return out, br
    return out


# revision 21
# speedup vs baseline: 1.0286x; 1.0286x over previous
"""LIF neuron scan kernel for Trainium2, sharded over 8 NeuronCores.

Device computes the membrane trajectory with ONE fused custom-DVE
instruction per time step (a microprogrammed 8-slice ALU chain, table
written per-NEFF — no firmware change):

    u_t = select(u_{t-1} < 1, u_{t-1}, 0) * 0.95 + x_t

i.e. the state is carried as the pre-reset potential u, and the reset,
decay and input-add all happen inside one DVE pass (1 elem/cycle/lane,
~242ns per [128,128] step tile vs 2-3 stock instructions).

The decay uses a single-rounded 0.95 multiply; the reference rounds twice
(v - v*0.05). Measured divergence ("drift") of the device trajectory from
the exact one is <= ~4e-6 (differences reset to zero whenever both
trajectories spike together). Host post-pass repairs any possible spike
flips — see the q-code scheme below.

Output: the otherwise-idle ACT engine quantizes u to a uint8 near-threshold
code per chunk (round-to-nearest, saturating — HW-verified):

    q = sat_u8(K*(u - 1) + 128),  K = 16384

so q >= 128  <=>  u >= 1 - 0.5/K   (the spike bit, exact outside the band)
and q in [125, 131]  <=>  |u - 1| <~ 2.1e-4  (the repair band, 50x drift).
This cuts output DMA 4x (26.2MB -> 6.55MB per core), putting total HBM
traffic at 32.8MB/core, under the ~358GB/s HBM-per-core roofline at the
~120us target. Host: spikes = (q >= 128); rows with any q in [125,131]
are re-simulated exactly (numpy f32, reference op order) and replaced —
~1-3% of rows.

Sharding: batch dim B=131072 split into 8 contiguous blocks of 16384 rows.
Per core the block is laid out time-major as [128 partitions, 400 steps,
128 neurons]; each step is one [128,128] SBUF tile; u is written in place
over the input tile (the previous step's tile is the recurrence input).

Raw Bass (no TileContext), semaphores at per-chunk granularity:
  dma_in   (+16 per input DMA, sync queue)   gates DVE's first read
  dma_in2  (+16 per input DMA, scalar queue) same, for the early chunk
                                             issued from the scalar queue
  dve_done (+1 per chunk)   gates the ACT quantize
  act_done (+1 per chunk)   gates the output DMA (dma_start is
                            sequencer-only and does NOT order after
                            same-engine compute writes) + xin reuse
  dma_out  (+16 per output DMA, scalar queue) gates sout reuse + kernel end
All intra-engine ordering is program order (engines execute in-order).
"""

import numpy as np

import concourse.bacc as bacc
import concourse.mybir as mybir
from concourse.bass_utils import run_bass_kernel_spmd

B, L = 131072, 400
NCORES = 8
RPC = B // NCORES      # rows (neurons) per core
P = 128                # SBUF partitions
J = RPC // P           # neurons per partition = 128 (one step = [P, J] tile)
# Chunk schedule: small first chunks to fill the pipe fast, small last to
# drain fast. Sums to L.
# Big body chunks: each chunk boundary exposes the ~2.3us DMA completion
# ack on the DVE critical path (the input stream's transfer time per chunk
# nearly equals the DVE chunk time — the kernel sits at the HBM roofline —
# so there is no pipeline slack to absorb it). 60-step chunks cut the
# number of exposures and amortize transfer overheads; small first chunks
# fill the pipe fast, small last drain fast. Body-chunk input DMAs are
# additionally split A(16 steps)+B(rest) with separate waits, so the
# chunk-start wait covers only the small leading transfer and B's ack
# hides behind the first 16 steps of compute.
CHUNKS = [4, 8, 12, 16, 20] + [60] * 5 + [16, 12, 8, 4]
assert sum(CHUNKS) == L
TCMAX = max(CHUNKS)
SUBA = 16              # leading sub-DMA steps for chunks with TC > SUBA+8
NBUF = 4               # in/out chunk buffers (60-step tiles: 4x30KB xin +
                       # 4x7.5KB sout = 150KB/partition of SBUF)
# Chunk 1's input DMA issues from the (initially idle) scalar queue so its
# descriptor generation overlaps chunk 0's on the sync queue.
SCALAR_IN_CHUNKS = {1}

DECAY = 0.95           # single-rounded decay multiplier (host repairs)
QK = 16384.0           # u8 code scale: q = sat_u8(QK*(u-1) + 128)
QLO, QHI = 125, 131    # repair band in q units (|u-1| <~ 2.1e-4)

_nc_cache = None
_lif_op = None


def _get_lif_op():
    """Register the fused LIF step as a custom DVE op (idempotent)."""
    global _lif_op
    if _lif_op is not None:
        return _lif_op
    from concourse.dve_ops import (
        CUSTOM_DVE_SPECS,
        OPS,
        _SUB_OPCODE_FOR_NAME,
        DveOp,
    )
    from concourse.dve_spec import C0, One, Spec, Src0, Src1, Zero, lower, select
    from concourse.dve_uop import DveOpSpec

    name = "LIF_STEP_ANT"
    if name in _SUB_OPCODE_FOR_NAME:
        _lif_op = next(op for op in OPS if op.name == name)
        return _lif_op
    spec = Spec(
        body=select(Src0 < One, Src0, Zero) * C0 + Src1,
        reference=lambda in0, in1, s0, s1, imm2: np.where(in0 < 1.0, in0, 0.0)
        .astype(np.float32)
        * np.float32(s0)
        + in1,
    )
    row = max(_SUB_OPCODE_FOR_NAME.values()) + 1
    assert row < 0x20
    shas = {}
    for ver in ("v3", "v4"):
        try:
            s = DveOpSpec(name=name, opcode=row, uops=lower(spec, ver=ver), rd1_en=True)
            shas[ver] = s.sha(ver)
        except Exception:
            pass
    op = DveOp(name, spec, subdim=False, uops_sha=shas)
    OPS.append(op)
    _SUB_OPCODE_FOR_NAME[name] = row
    CUSTOM_DVE_SPECS[name] = spec
    _lif_op = op
    return op


def _build():
    lif = _get_lif_op()
    nc = bacc.Bacc(None, target_bir_lowering=False)
    X = nc.dram_tensor("X", [P, L * J], mybir.dt.float32, kind="ExternalInput")
    S = nc.dram_tensor("S", [P, L * J], mybir.dt.uint8, kind="ExternalOutput")

    f32 = mybir.dt.float32
    xin = [nc.alloc_sbuf_tensor(f"xin{i}", [P, TCMAX * J], f32) for i in range(NBUF)]
    sout = [
        nc.alloc_sbuf_tensor(f"sout{i}", [P, TCMAX * J], mybir.dt.uint8)
        for i in range(NBUF)
    ]
    zt = nc.alloc_sbuf_tensor("zt", [P, J], f32)   # u_{-1} = 0

    # Input-DMA completion tracking: a +16-per-DMA cumulative count is only
    # sound if same-semaphore DMAs complete in order (the 16 SDMA engines
    # finish their partition slices independently, so two in-flight DMAs on
    # one count can interleave incs and a wait on 16*n passes with DMA n
    # still draining on a laggard engine — observed as stale x on single
    # partitions). Chunks rotate over NSLOT semaphores; each slot chains on
    # its own previous DMA, bounding in-flight DMAs per slot to 1 (exact
    # count) while keeping NSLOT transfers in flight overall.
    # 2 slots: enough overlap to hide the ~2.3us per-DMA completion ack
    # (cadence ~3us < the 4.8us DVE chunk pace) while capping concurrent
    # input streams — 4 slots measurably slowed the DVE (~+40ns/op) via
    # SBUF write-port contention from the extra in-flight transfers.
    NSLOT = 2
    sem_in = [nc.alloc_semaphore(f"dma_slot{k}") for k in range(NSLOT)]
    sem_dma_in2 = nc.alloc_semaphore("dma_in2")   # scalar-queue input DMAs
    sem_dve = nc.alloc_semaphore("dve_done")
    sem_act = nc.alloc_semaphore("act_done")
    # Output-DMA completions use the same slot-pool scheme as inputs (2
    # slots): per-slot self-chaining keeps each cumulative count exact
    # while letting two transfers overlap, and the chain wait (own slot's
    # previous DMA, 2 chunks back) is long satisfied by the time the ACT
    # sequencer reaches it — no stall.
    NOSLOT = 2
    sem_out = [nc.alloc_semaphore(f"dma_oslot{k}") for k in range(NOSLOT)]

    # Chunk base offsets (in steps).
    bases = []
    t0 = 0
    for tc in CHUNKS:
        bases.append(t0)
        t0 += tc

    # --- input DMAs, NBUF-deep rolling prefetch --------------------------
    # DMA for chunk c overwrites xin[c%NBUF]; its last readers are chunk
    # c-NBUF's ACT quantize and chunk c-NBUF+1's first LIF op (which reads
    # chunk c-NBUF's final u tile).
    assert all(c < NBUF for c in SCALAR_IN_CHUNKS)
    in_rank = {}      # chunk -> list of (sem, cumulative wait value, step)
    slot_cnt = [0] * NSLOT
    nsc = 0
    nseq = 0          # sequential index over sync-queue input DMAs

    def _issue(dst, src, s):
        if slot_cnt[s] > 0:
            # Chain on this slot's previous DMA (exact per-slot count).
            nc.sync.wait_ge(sem_in[s], 16 * slot_cnt[s])
        slot_cnt[s] += 1
        nc.sync.dma_start(dst, src).then_inc(sem_in[s], 16)
        return (sem_in[s], 16 * slot_cnt[s])

    for c, TC in enumerate(CHUNKS):
        base = bases[c] * J
        if c in SCALAR_IN_CHUNKS:
            nsc += 1
            in_rank[c] = [(sem_dma_in2, 16 * nsc, 0)]
            nc.scalar.dma_start(
                xin[c % NBUF][:, : TC * J], X[:, base : base + TC * J]
            ).then_inc(sem_dma_in2, 16)
            continue
        if c >= NBUF:
            # xin[c%NBUF] reuse gates.
            nc.sync.wait_ge(sem_act, c - NBUF + 1)
            nc.sync.wait_ge(sem_dve, c - NBUF + 2)
        xb = xin[c % NBUF]
        if TC > SUBA + 8:
            sa, va = _issue(
                xb[:, : SUBA * J], X[:, base : base + SUBA * J], nseq % NSLOT
            )
            nseq += 1
            sb_, vb = _issue(
                xb[:, SUBA * J : TC * J],
                X[:, base + SUBA * J : base + TC * J],
                nseq % NSLOT,
            )
            nseq += 1
            in_rank[c] = [(sa, va, 0), (sb_, vb, SUBA)]
        else:
            s_, v_ = _issue(
                xb[:, : TC * J], X[:, base : base + TC * J], nseq % NSLOT
            )
            nseq += 1
            in_rank[c] = [(s_, v_, 0)]

    # --- DVE queue: the full recurrence, one fused op per step -----------
    nc.vector.memset(zt[:], 0.0)

    prev = zt[:, :]
    for c, TC in enumerate(CHUNKS):
        xb = xin[c % NBUF]
        waits = {step: (ws, wv) for ws, wv, step in in_rank[c]}
        for t in range(TC):
            sl = slice(t * J, (t + 1) * J)
            if t in waits:
                # Gate reads on the covering input (sub-)DMA.
                ws, wv = waits[t]
                nc.vector.wait_ge(ws, wv)
            # u_t = select(u_{t-1} < 1, u_{t-1}, 0) * 0.95 + x_t  (in place)
            ri = nc.vector._custom_dve(
                lif, out=xb[:, sl], in0=prev, in1=xb[:, sl], s0=DECAY
            )
            prev = xb[:, sl]
        # Last DVE op of the chunk certifies all u tiles of xin[c%NBUF].
        ri.then_inc(sem_dve, 1)

    # --- ACT queue: u8 quantize + output DMA -----------------------------
    ocnt = [0] * NOSLOT
    orank = {}        # chunk -> (slot, count)
    for c, TC in enumerate(CHUNKS):
        xb = xin[c % NBUF]
        sb = sout[c % NBUF]
        nc.scalar.wait_ge(sem_dve, c + 1)
        if c >= NBUF:
            # sout[c%NBUF] reuse: chunk c-NBUF's out-DMA must have drained.
            ps, pk = orank[c - NBUF]
            nc.scalar.wait_ge(sem_out[ps], 16 * pk)
        nc.scalar.activation(
            sb[:, : TC * J],
            xb[:, : TC * J],
            mybir.ActivationFunctionType.Copy,
            bias=128.0 - QK,
            scale=QK,
        ).then_inc(sem_act, 1)
        nc.scalar.wait_ge(sem_act, c + 1)
        s = c % NOSLOT
        if ocnt[s] > 0:
            # Chain on this slot's previous DMA (exact per-slot count).
            nc.scalar.wait_ge(sem_out[s], 16 * ocnt[s])
        ocnt[s] += 1
        orank[c] = (s, ocnt[s])
        base = bases[c] * J
        nc.scalar.dma_start(S[:, base : base + TC * J], sb[:, : TC * J]).then_inc(
            sem_out[s], 16
        )

    # Hold kernel end until the last output DMAs drained, then zero the
    # semaphores so back-to-back NEFF executions see a clean file.
    for s in range(NOSLOT):
        nc.sync.wait_ge(sem_out[s], 16 * ocnt[s])
    for s in (*sem_in, sem_dma_in2, sem_dve, sem_act, *sem_out):
        nc.sync.sem_clear(s)

    nc.compile()
    return nc


def _get_nc():
    global _nc_cache
    if _nc_cache is None:
        _nc_cache = _build()
    return _nc_cache


def _shard(I):
    # Per-core host transposes run in parallel (numpy releases the GIL
    # during the strided copies).
    from concurrent.futures import ThreadPoolExecutor

    def one(c):
        Ic = I[c * RPC : (c + 1) * RPC]                    # [RPC, L]
        Xc = Ic.reshape(P, J, L).transpose(0, 2, 1)        # [P, L, J] time-major
        return {"X": np.ascontiguousarray(Xc).reshape(P, L * J)}

    with ThreadPoolExecutor(NCORES) as ex:
        return list(ex.map(one, range(NCORES)))


def _unshard(results):
    from concurrent.futures import ThreadPoolExecutor

    out = np.empty((B, L), np.uint8)

    def one(c):
        Sc = results[c]["S"].reshape(P, L, J).transpose(0, 2, 1)   # [P, J, L]
        out[c * RPC : (c + 1) * RPC] = Sc.reshape(RPC, L)

    with ThreadPoolExecutor(NCORES) as ex:
        list(ex.map(one, range(NCORES)))
    return out


def _resim_exact(Irows):
    """Exact reference recurrence (numpy f32, same rounding order as the
    jax/XLA raster: u = (v - v*0.05) + I_t)."""
    n = Irows.shape[0]
    v = np.zeros(n, np.float32)
    s = np.empty((n, L), np.float32)
    c05 = np.float32(0.05)
    one = np.float32(1.0)
    for t in range(L):
        u = (v - v * c05) + Irows[:, t]
        spk = u >= one
        s[:, t] = spk
        v = np.where(spk, np.float32(0.0), u)
    return s


def _decode(q, I):
    """q codes -> spike raster, with exact re-simulation of band rows."""
    from concurrent.futures import ThreadPoolExecutor

    out = np.empty((B, L), np.float32)
    flagged = []

    def one(c):
        lo, hi = c * RPC, (c + 1) * RPC
        qc = q[lo:hi]
        out[lo:hi] = (qc >= 128).astype(np.float32)
        near = (qc >= QLO) & (qc <= QHI)
        return np.nonzero(near.any(axis=1))[0] + lo

    with ThreadPoolExecutor(NCORES) as ex:
        for rows in ex.map(one, range(NCORES)):
            flagged.append(rows)
    flagged = np.concatenate(flagged)
    if flagged.size:
        out[flagged] = _resim_exact(I[flagged])
    return out, flagged


def kernel(I, _trace=False, _debug=False):
    I = np.ascontiguousarray(np.asarray(I), dtype=np.float32)
    assert I.shape == (B, L), I.shape
    nc = _get_nc()
    br = run_bass_kernel_spmd(nc, _shard(I), core_ids=list(range(NCORES)), trace=_trace)
    q = _unshard(br.results)
    out, flagged = _decode(q, I)
    if _debug:
        return out, q, flagged
    if _trace:
        return out, br
    return out
